# revision 24
# baseline (speedup 1.0000x reference)
"""Trainium2 Bass kernel for nn_AdaptiveBlock (dense_mlp).

Reference computation:
    y    = mean(x, axis=(2, 3))                   # (B, C) global avg pool
    h    = gelu(y @ W1)                           # (B, HID), exact erf gelu
    yp   = gelu(h @ W2)                           # (B, C)
    A    = yp @ WA + bA                           # (B, H)
    Bv   = yp @ WB + bB                           # (B, W)
    attn = sigmoid(A[:,None,:,None] * Bv[:,None,None,:])   # (B, 1, H, W)
    out  = broadcast(attn, (B, C, H, W))

Sharding: data-parallel over batch across 8 NeuronCores (4 batches/core),
weights replicated, no collectives.  Each core streams its 51.4 MB x-shard
through SBUF and row-reduces it on VectorE (overlapped with DMA), then runs
the tiny MLP epilogue on TensorE/ScalarE (bf16 compute, f32 accumulate) and
writes out only its (4, 56*56) attention map.  The channel broadcast is
done on the host (it carries no information).

Everything is raw Bass with hand-rolled semaphores: the pinned walrus only
accepts a single sync-wait per DMA/LDWEIGHTS instruction, which rules out
Tile's generated multi-wait instructions for this pipeline shape.  Raw mode
uses standalone wait_ge instructions instead.

Structure per core:
  phase 1 (streaming): x DMAs on the SP HWDGE ring in tiles (mostly 4x128
    rows x 3136, with a smaller tail so the last reduce is short), VectorE
    reduce_sum per tile.  Weight DMAs (SWDGE, casting f32->bf16 in flight)
    are gated on reduce progress so they don't steal HBM bandwidth from the
    x stream until it is nearly done.  GpSimd builds the identity matrix and
    the block-diagonal mask; ScalarE preloads the Gelu ACT table.
  phase 2 (epilogue): mm1 -> gelu -> PE-transpose -> mm2 -> gelu ->
    PE-transpose -> [A|Bv] matmul (+bias via ones-row matmul) -> outer
    product as one PE matmul against a block-diagonal Bv matrix (built with
    one masked VectorE multiply) -> sigmoid on (56, 224) -> DMA out.
"""

import numpy as np

import concourse.bass as bass
from concourse import mybir
from concourse.bass_utils import run_bass_kernel_spmd

B, C, HID, H, W = 32, 1024, 512, 56, 56
NCORES = 8
BS = B // NCORES          # 4 batches per core
ROWS = BS * C             # 4096 (b, c) rows per core
HW = H * W                # 3136
NBLK = ROWS // 128        # 32 row-blocks of 128
# per-DMA-tile block counts; small tail so the final reduce is short
TILE_SIZES = [8, 8, 8, 4, 2, 1, 1]
assert sum(TILE_SIZES) == NBLK
NT = len(TILE_SIZES)
SLOT_BLKS = max(TILE_SIZES)   # buffer slot capacity (blocks)
NBUF = 2                      # x double-buffer slots
F32 = mybir.dt.float32
BF16 = mybir.dt.bfloat16


def build_bass(gelu_fn=None, debug_taps=False) -> bass.Bass:
    # gelu_fn override exists only so CoreSim (which lacks a Gelu impl) can
    # check the dataflow with a substitute activation.
    if gelu_fn is None:
        gelu_fn = mybir.ActivationFunctionType.Gelu
    nc = bass.Bass()

    x_t = nc.dram_tensor("x", [ROWS, HW], BF16, kind="ExternalInput")
    w1_t = nc.dram_tensor("W1bf", [C, HID], BF16, kind="ExternalInput")
    w2_t = nc.dram_tensor("W2bf", [HID, C], BF16, kind="ExternalInput")
    wa_t = nc.dram_tensor("WAbf", [C, H], BF16, kind="ExternalInput")
    ba_t = nc.dram_tensor("bAbf", [H], BF16, kind="ExternalInput")
    wb_t = nc.dram_tensor("WBbf", [C, W], BF16, kind="ExternalInput")
    bb_t = nc.dram_tensor("bBbf", [W], BF16, kind="ExternalInput")
    out_t = nc.dram_tensor("out", [BS, HW], F32, kind="ExternalOutput")
    dbg = {}
    if debug_taps:
        for name, shape in [
            ("dbg_ysum", [128, NBLK]), ("dbg_ysum_bf", [128, NBLK]),
            ("dbg_h", [BS, HID]), ("dbg_yp", [BS, C]),
            ("dbg_ab", [BS, H + W]), ("dbg_bdiag", [BS, BS * W]),
            ("dbg_at", [H, BS * W]),
        ]:
            dbg[name] = nc.dram_tensor(name, shape, F32, kind="ExternalOutput")

    # x row r = b*C + c; block j = r // 128 = b*8 + c_chunk.  Tile n covers
    # blocks [off_n, off_n + s_n); ysum column j holds sum of row block j.
    x_blk = x_t[:, :].rearrange("(j p) m -> j p m", p=128)
    offs = [sum(TILE_SIZES[:n]) for n in range(NT)]

    # Block-reduce ownership: DVE uses scalar_tensor_tensor as a fused
    # pairwise-add + accumulate (one op per block, pays free-dim cycles for
    # 1568 outputs); ACT reduces a few blocks via activation(Copy,
    # accum_out) at ~3.5us/block to add slack.
    OWNER = ["A" if j % 16 in (3, 8, 13) else "D" for j in range(NBLK)]
    # cumulative per-owner block counts through tile t
    cumD = [sum(1 for j in range(offs[t] + TILE_SIZES[t]) if OWNER[j] == "D")
            for t in range(NT)]
    cumA = [sum(1 for j in range(offs[t] + TILE_SIZES[t]) if OWNER[j] == "A")
            for t in range(NT)]

    # ---- SBUF ----
    x_sb = nc.alloc_sbuf_tensor("x_sb", [128, NBUF, SLOT_BLKS, HW], BF16)
    # throwaway elementwise outputs of the accumulate-reduces (only
    # accum_out matters); per-engine ops serialize so one scratch each
    ascr_sb = nc.alloc_sbuf_tensor("ascr_sb", [128, HW], BF16)
    dscr_sb = nc.alloc_sbuf_tensor("dscr_sb", [128, HW // 2], BF16)
    ysum_sb = nc.alloc_sbuf_tensor("ysum_sb", [128, NBLK], F32)
    ysum_bf = nc.alloc_sbuf_tensor("ysum_bf", [128, NBLK], BF16)
    w1_sb = nc.alloc_sbuf_tensor("w1_sb", [128, C // 128, HID], BF16)
    w2_sb = nc.alloc_sbuf_tensor("w2_sb", [128, HID // 128, C], BF16)
    wab_sb = nc.alloc_sbuf_tensor("wab_sb", [128, C // 128, H + W], BF16)
    bab_sb = nc.alloc_sbuf_tensor("bab_sb", [1, H + W], BF16)
    ident_sb = nc.alloc_sbuf_tensor("ident_sb", [128, 128], BF16)
    ones_sb = nc.alloc_sbuf_tensor("ones_sb", [1, BS], BF16)
    mask_sb = nc.alloc_sbuf_tensor("mask_sb", [BS, BS, W], BF16)
    h_sb = nc.alloc_sbuf_tensor("h_sb", [BS, HID], BF16)
    hT_sb = nc.alloc_sbuf_tensor("hT_sb", [128, (HID // 128) * BS], BF16)
    yp_sb = nc.alloc_sbuf_tensor("yp_sb", [BS, C], BF16)
    ypT_sb = nc.alloc_sbuf_tensor("ypT_sb", [128, (C // 128) * BS], BF16)
    ab_sb = nc.alloc_sbuf_tensor("ab_sb", [BS, H + W], BF16)
    bdiag_sb = nc.alloc_sbuf_tensor("bdiag_sb", [BS, BS, W], BF16)
    attn_sb = nc.alloc_sbuf_tensor("attn_sb", [H, BS, W], F32)
    scr_sb = nc.alloc_sbuf_tensor("scr_sb", [1, 1], F32)

    # ---- PSUM (each tensor gets its own 2KB bank; 7 of 8 banks used) ----
    ps_h = nc.alloc_psum_tensor("ps_h", [BS, HID], F32)
    ps_yp1 = nc.alloc_psum_tensor("ps_yp1", [BS, C // 2], F32)
    ps_yp2 = nc.alloc_psum_tensor("ps_yp2", [BS, C // 2], F32)
    ps_ab = nc.alloc_psum_tensor("ps_ab", [BS, H + W], F32)
    ps_at = nc.alloc_psum_tensor("ps_at", [H, BS, W], F32)
    ps_warm = nc.alloc_psum_tensor("ps_warm", [BS, 128], F32)
    # two transpose scratch banks, ping-pong so PE-write and DVE-read never
    # touch the same PSUM bank concurrently (HW fault otherwise)
    tp_banks = [
        nc.alloc_psum_tensor("tp_a", [128, BS], BF16),
        nc.alloc_psum_tensor("tp_b", [128, BS], BF16),
    ]

    # ---- semaphores ----
    # One semaphore per DMA: with several DMAs in flight on one counting
    # semaphore, per-SDMA-engine increments can interleave and a cumulative
    # wait_ge would not imply the earlier DMA fully landed.
    xdma_sems = [nc.alloc_semaphore(f"xdma_sem{n}") for n in range(NT)]
    w_sems = [nc.alloc_semaphore(f"w_sem{i}") for i in range(6)]
    id_sem = nc.alloc_semaphore("id_sem")
    ones_sem = nc.alloc_semaphore("ones_sem")
    red_d = nc.alloc_semaphore("red_d")
    red_a = nc.alloc_semaphore("red_a")
    pe_sem = nc.alloc_semaphore("pe_sem")
    act_sem = nc.alloc_semaphore("act_sem")
    dve_sem = nc.alloc_semaphore("dve_sem")
    out_sem = nc.alloc_semaphore("out_sem")
    out2_sem = nc.alloc_semaphore("out2_sem")

    NCC = C // 128    # 8
    NQH = HID // 128  # 4
    # start weight DMAs early-mid-stream (total HBM bytes are fixed; they
    # just must finish before the x tail so they don't delay mm1); counted
    # in DVE-owned blocks
    W_GATE = 4

    # lhsT for chunk cc: ysum columns j = b*8 + cc, b = 0..3
    ysum_r = ysum_bf[:, :].rearrange("p (b c) -> p c b", b=BS)

    # Single Block: a Block exit drains every engine (including waiting for
    # outstanding SWDGE DMA completions) and cross-syncs, so phase
    # boundaries inside the kernel would serialize the pipeline.  All
    # ordering is via explicit semaphores instead.
    with nc.Block() as blk:

        @blk.sync
        def _(sync):
            for n in range(NT):
                if n >= NBUF:
                    # slot reuse: all blocks of tile n-NBUF must be reduced
                    sync.wait_ge(red_d, cumD[n - NBUF])
                    sync.wait_ge(red_a, cumA[n - NBUF])
                sync.dma_start(
                    out=x_sb[:, n % NBUF, 0 : TILE_SIZES[n], :],
                    in_=x_blk[offs[n] : offs[n] + TILE_SIZES[n]].rearrange(
                        "j p m -> p j m"
                    ),
                ).then_inc(xdma_sems[n], 16)
            out_r = out_t[:, :].rearrange("b (h w) -> h b w", h=H)
            sync.wait_ge(act_sem, 5)
            sync.dma_start(
                out=out_r[0 : 32], in_=attn_sb[0 : 32, :, :]
            ).then_inc(out_sem, 16)
            sync.wait_ge(act_sem, 6)
            sync.dma_start(
                out=out_r[32 : H], in_=attn_sb[32 : H, :, :]
            ).then_inc(out2_sem, 16)
            sync.wait_ge(out_sem, 16)
            sync.wait_ge(out2_sem, 16)

        @blk.vector
        def _(vec):
            vec.memset(ones_sb[:, :], 1.0).then_inc(ones_sem, 1)
            for n in range(NT):
                if not any(OWNER[offs[n] + k] == "D" for k in range(TILE_SIZES[n])):
                    continue
                vec.wait_ge(xdma_sems[n], 16)
                for k in range(TILE_SIZES[n]):
                    j = offs[n] + k
                    if OWNER[j] != "D":
                        continue
                    nc.vector.scalar_tensor_tensor(
                        out=dscr_sb[:, :],
                        in0=x_sb[:, n % NBUF, k, 0 : HW // 2],
                        scalar=0.0,
                        in1=x_sb[:, n % NBUF, k, HW // 2 : HW],
                        op0=mybir.AluOpType.add,
                        op1=mybir.AluOpType.add,
                        accum_out=ysum_sb[:, j : j + 1],
                    ).then_inc(red_d, 1)
            for q in range(NQH):
                vec.wait_ge(pe_sem, 9 + q)
                nc.vector.tensor_copy(
                    out=hT_sb[:, q * BS : (q + 1) * BS],
                    in_=tp_banks[q % 2][:, :],
                ).then_inc(dve_sem, 1)
            for q in range(NCC):
                vec.wait_ge(pe_sem, 21 + q)
                nc.vector.tensor_copy(
                    out=ypT_sb[:, q * BS : (q + 1) * BS],
                    in_=tp_banks[q % 2][:, :],
                ).then_inc(dve_sem, 1)
            vec.wait_ge(pe_sem, 37)
            nc.vector.tensor_copy(
                out=ab_sb[:, :], in_=ps_ab[:, :]
            ).then_inc(dve_sem, 1)
            vec.wait_ge(dve_sem, 1 + NQH + NCC)
            vec.wait_ge(id_sem, 4)
            # bdiag[b, bb, w] = Bv[b, w] * (b == bb)
            b_sl = ab_sb[:, H : H + W]
            b_bc = bass.AP(
                tensor=b_sl.tensor, offset=b_sl.offset,
                ap=[b_sl.ap[0], [0, BS], [b_sl.ap[1][0], W]],
            )
            nc.vector.tensor_mul(
                out=bdiag_sb[:, :, :], in0=b_bc, in1=mask_sb[:, :, :]
            ).then_inc(dve_sem, 1)

        @blk.gpsimd
        def _(gpsimd):
            # the 8 Q7 cores don't serialize consecutive Pool ops, so sync
            # memset -> affine_select explicitly
            gpsimd.memset(ident_sb[:, :], 0.0).then_inc(id_sem, 1)
            gpsimd.memset(mask_sb[:, :, :], 0.0).then_inc(id_sem, 1)
            gpsimd.wait_ge(id_sem, 2)
            gpsimd.affine_select(
                out=ident_sb[:, :],
                in_=ident_sb[:, :],
                compare_op=mybir.AluOpType.not_equal,
                fill=1.0,
                base=0,
                pattern=[[-1, 128]],
                channel_multiplier=1,
            ).then_inc(id_sem, 1)
            # mask[p, bb, w] = (p == bb) ? 1 : 0
            gpsimd.affine_select(
                out=mask_sb[:, :, :],
                in_=mask_sb[:, :, :],
                compare_op=mybir.AluOpType.not_equal,
                fill=1.0,
                base=0,
                pattern=[[-1, BS], [0, W]],
                channel_multiplier=1,
            ).then_inc(id_sem, 1)
            # weight loads (already bf16 from the host) on the idle GpSimd
            # SWDGE ring; gated so they don't compete with the x stream
            # until it is nearly done
            gpsimd.wait_ge(red_d, W_GATE)
            w_loads = [
                (w1_sb[:, :, :],
                 w1_t[:, :].rearrange("(n p) h -> p n h", p=128)),
                (w2_sb[:, :, :],
                 w2_t[:, :].rearrange("(n p) h -> p n h", p=128)),
                (wab_sb[:, :, 0:H],
                 wa_t[:, :].rearrange("(n p) h -> p n h", p=128)),
                (wab_sb[:, :, H : H + W],
                 wb_t[:, :].rearrange("(n p) h -> p n h", p=128)),
                (bab_sb[0:1, 0:H], ba_t[None, :]),
                (bab_sb[0:1, H : H + W], bb_t[None, :]),
            ]
            for i, (dst, src) in enumerate(w_loads):
                gpsimd.dma_start(out=dst, in_=src).then_inc(w_sems[i], 16)
            if debug_taps:
                gpsimd.wait_ge(act_sem, 6)
                taps = [
                    (dbg["dbg_ysum"], ysum_sb[:, :]),
                    (dbg["dbg_ysum_bf"], ysum_bf[:, :]),
                    (dbg["dbg_h"], h_sb[:, :]),
                    (dbg["dbg_yp"], yp_sb[:, :]),
                    (dbg["dbg_ab"], ab_sb[:, :]),
                    (dbg["dbg_bdiag"],
                     bdiag_sb[:, :, :].rearrange("b bb w -> b (bb w)")),
                    (dbg["dbg_at"],
                     attn_sb[:, :, :].rearrange("h b w -> h (b w)")),
                ]
                dbg_sem = nc.alloc_semaphore("dbg_sem")
                for i, (dst, src_ap) in enumerate(taps):
                    gpsimd.dma_start(out=dst[:, :], in_=src_ap).then_inc(
                        dbg_sem, 16
                    )
                    gpsimd.wait_ge(dbg_sem, 16 * (i + 1))

        # PE ticks (every non-dummy PE op +1 on pe_sem):
        #  1..8  mm1        9..12  transpose h    13..20 mm2 (yp1@19, yp2@20)
        #  21..28 tr yp     29..37 mm3 + bias     38 outer product
        # ACT: ysum cast 1, gelu_h 2, gelu_yp1 3, gelu_yp2 4, sigmoid 5
        # DVE: hT copies 1..4, ypT copies 5..12, ab copy 13, bdiag mul 14
        # id_sem: ident memset/select + mask memset/select (GpSimd) + ones

        @blk.tensor
        def _(pe):
            pe.wait_ge(id_sem, 4)
            pe.wait_ge(ones_sem, 1)
            # warm the PE clock (HAM): opportunistic burst near stream
            # end, then a full 3.4us HAM-window burst paced by the last
            # x-DMA landing -- it runs exactly during the final reduce +
            # cast, so mm1 always starts warm and undelayed
            pe.wait_ge(red_d, cumD[NT - 2])
            for _i in range(24):
                nc.tensor.matmul(
                    ps_warm[:, :], ident_sb[:, 0:BS], ident_sb[:, :],
                    start=True, stop=True,
                )
            pe.wait_ge(xdma_sems[NT - 1], 16)
            for _i in range(16):
                nc.tensor.matmul(
                    ps_warm[:, :], ident_sb[:, 0:BS], ident_sb[:, :],
                    start=True, stop=True,
                )
            pe.wait_ge(act_sem, 1)
            pe.wait_ge(w_sems[0], 16)
            for cc in range(NCC):
                nc.tensor.matmul(
                    ps_h[:, :],
                    ysum_r[:, cc, :],
                    w1_sb[:, cc, :],
                    start=(cc == 0),
                    stop=(cc == NCC - 1),
                ).then_inc(pe_sem, 1)
            pe.wait_ge(act_sem, 2)
            for q in range(NQH):
                if q >= 2:
                    pe.wait_ge(dve_sem, q - 1)
                nc.tensor.transpose(
                    tp_banks[q % 2][:, :],
                    h_sb[:, q * 128 : (q + 1) * 128],
                    ident_sb[:BS, :BS],
                ).then_inc(pe_sem, 1)
            pe.wait_ge(w_sems[1], 16)
            for q in range(NQH):
                pe.wait_ge(dve_sem, q + 1)
                lhsT = hT_sb[:, q * BS : (q + 1) * BS]
                nc.tensor.matmul(
                    ps_yp1[:, :], lhsT, w2_sb[:, q, 0 : C // 2],
                    start=(q == 0), stop=(q == NQH - 1),
                ).then_inc(pe_sem, 1)
                nc.tensor.matmul(
                    ps_yp2[:, :], lhsT, w2_sb[:, q, C // 2 : C],
                    start=(q == 0), stop=(q == NQH - 1),
                ).then_inc(pe_sem, 1)
            pe.wait_ge(act_sem, 3)
            for q in range(NCC):
                if q == NQH:
                    pe.wait_ge(act_sem, 4)
                if q >= 2:
                    pe.wait_ge(dve_sem, q + 3)
                nc.tensor.transpose(
                    tp_banks[q % 2][:, :],
                    yp_sb[:, q * 128 : (q + 1) * 128],
                    ident_sb[:BS, :BS],
                ).then_inc(pe_sem, 1)
            pe.wait_ge(w_sems[2], 16)
            pe.wait_ge(w_sems[3], 16)
            for cc in range(NCC):
                pe.wait_ge(dve_sem, NQH + 1 + cc)
                nc.tensor.matmul(
                    ps_ab[:, :],
                    ypT_sb[:, cc * BS : (cc + 1) * BS],
                    wab_sb[:, cc, :],
                    start=(cc == 0),
                    stop=False,
                ).then_inc(pe_sem, 1)
            pe.wait_ge(w_sems[4], 16)
            pe.wait_ge(w_sems[5], 16)
            nc.tensor.matmul(
                ps_ab[:, :], ones_sb[:, :], bab_sb[:, :],
                start=False, stop=True,
            ).then_inc(pe_sem, 1)
            # outer products: out[h, (b w)] = sum_b' A[b', h] * bdiag[b', (b w)]
            pe.wait_ge(dve_sem, 2 + NQH + NCC)
            nc.tensor.matmul(
                ps_at[:, :, :].rearrange("h b w -> h (b w)"),
                ab_sb[:, 0:H],
                bdiag_sb[:, :, :].rearrange("b bb w -> b (bb w)"),
                start=True, stop=True,
            ).then_inc(pe_sem, 1)

        @blk.scalar
        def _(act):
            # dummy activation so walrus loads the Gelu ACT table here, early
            # and in the same basic block as the real gelus (no reload)
            zero = nc.const_aps.aps[(F32, 0.0)]
            nc.scalar.activation(scr_sb[0:1, :], zero[0:1, :], gelu_fn)
            # ACT's share of the block reduces (activation Copy + accum_out)
            for n in range(NT):
                if not any(OWNER[offs[n] + k] == "A" for k in range(TILE_SIZES[n])):
                    continue
                act.wait_ge(xdma_sems[n], 16)
                for k in range(TILE_SIZES[n]):
                    j = offs[n] + k
                    if OWNER[j] != "A":
                        continue
                    nc.scalar.activation(
                        out=ascr_sb[:, :],
                        in_=x_sb[:, n % NBUF, k, :],
                        func=mybir.ActivationFunctionType.Copy,
                        accum_out=ysum_sb[:, j : j + 1],
                    ).then_inc(red_a, 1)
            act.wait_ge(red_d, cumD[NT - 1])
            nc.scalar.copy(
                out=ysum_bf[:, :], in_=ysum_sb[:, :]
            ).then_inc(act_sem, 1)
            act.wait_ge(pe_sem, NCC)
            nc.scalar.activation(
                h_sb[:, :], ps_h[:, :], gelu_fn, scale=1.0 / HW
            ).then_inc(act_sem, 1)
            act.wait_ge(pe_sem, 8 + NQH + 2 * NQH - 1)   # yp1 done @19
            nc.scalar.activation(
                yp_sb[:, 0 : C // 2], ps_yp1[:, :], gelu_fn
            ).then_inc(act_sem, 1)
            act.wait_ge(pe_sem, 8 + NQH + 2 * NQH)       # yp2 done @20
            nc.scalar.activation(
                yp_sb[:, C // 2 : C], ps_yp2[:, :], gelu_fn
            ).then_inc(act_sem, 1)
            # dummy sigmoid so the ACT table switch happens off the
            # critical path, while the PE is still on transposes/mm3
            nc.scalar.activation(
                scr_sb[0:1, :], zero[0:1, :],
                mybir.ActivationFunctionType.Sigmoid,
            )
            # two halves so the first output DMA's HBM write receipt
            # overlaps the second half's sigmoid + transfer
            act.wait_ge(pe_sem, 38)
            nc.scalar.activation(
                attn_sb[0 : 32, :, :], ps_at[0 : 32, :, :],
                mybir.ActivationFunctionType.Sigmoid,
            ).then_inc(act_sem, 1)
            nc.scalar.activation(
                attn_sb[32 : H, :, :], ps_at[32 : H, :, :],
                mybir.ActivationFunctionType.Sigmoid,
            ).then_inc(act_sem, 1)

    return nc


_NC_CACHE: list = []


def run_on_hw(x, W1, W2, WA, bA, WB, bB, **spmd_kwargs):
    """Run the SPMD kernel; returns (full_output, BassKernelResults)."""
    import ml_dtypes

    bf = ml_dtypes.bfloat16
    # bf16 input stream: halves HBM traffic for the dominant x read; the
    # pooled-mean perturbation is ~0.6% of y's std, far inside tolerance
    x = np.ascontiguousarray(np.asarray(x, dtype=np.float32).astype(bf))
    weights = {
        "W1bf": np.ascontiguousarray(np.asarray(W1).astype(bf)),
        "W2bf": np.ascontiguousarray(np.asarray(W2).astype(bf)),
        "WAbf": np.ascontiguousarray(np.asarray(WA).astype(bf)),
        "bAbf": np.ascontiguousarray(np.asarray(bA).astype(bf)),
        "WBbf": np.ascontiguousarray(np.asarray(WB).astype(bf)),
        "bBbf": np.ascontiguousarray(np.asarray(bB).astype(bf)),
    }

    if not _NC_CACHE:
        _NC_CACHE.append(build_bass())
    nc = _NC_CACHE[0]

    in_maps = []
    for i in range(NCORES):
        shard = x[i * BS : (i + 1) * BS].reshape(ROWS, HW)
        in_maps.append({"x": shard, **weights})

    res = run_bass_kernel_spmd(
        nc, in_maps, core_ids=list(range(NCORES)), **spmd_kwargs
    )
    attn = np.concatenate([r["out"] for r in res.results], axis=0)  # (B, HW)
    return np.broadcast_to(attn.reshape(B, 1, H, W), (B, C, H, W)), res


def kernel(x, W1, W2, WA, bA, WB, bB):
    out, _ = run_on_hw(x, W1, W2, WA, bA, WB, bB)
    return out



# revision 29
# speedup vs baseline: 1.2001x; 1.2001x over previous
"""Trainium2 Bass kernel for nn_AdaptiveBlock (dense_mlp).

Reference computation:
    y    = mean(x, axis=(2, 3))                   # (B, C) global avg pool
    h    = gelu(y @ W1)                           # (B, HID), exact erf gelu
    yp   = gelu(h @ W2)                           # (B, C)
    A    = yp @ WA + bA                           # (B, H)
    Bv   = yp @ WB + bB                           # (B, W)
    attn = sigmoid(A[:,None,:,None] * Bv[:,None,None,:])   # (B, 1, H, W)
    out  = broadcast(attn, (B, C, H, W))

Sharding: data-parallel over batch across 8 NeuronCores (4 batches/core),
weights replicated, no collectives.  Each core streams its 51.4 MB x-shard
through SBUF and row-reduces it on VectorE (overlapped with DMA), then runs
the tiny MLP epilogue on TensorE/ScalarE (bf16 compute, f32 accumulate) and
writes out only its (4, 56*56) attention map.  The channel broadcast is
done on the host (it carries no information).

Everything is raw Bass with hand-rolled semaphores: the pinned walrus only
accepts a single sync-wait per DMA/LDWEIGHTS instruction, which rules out
Tile's generated multi-wait instructions for this pipeline shape.  Raw mode
uses standalone wait_ge instructions instead.

Structure per core:
  phase 1 (streaming): x DMAs on the SP HWDGE ring in tiles (mostly 4x128
    rows x 3136, with a smaller tail so the last reduce is short), VectorE
    reduce_sum per tile.  Weight DMAs (SWDGE, casting f32->bf16 in flight)
    are gated on reduce progress so they don't steal HBM bandwidth from the
    x stream until it is nearly done.  GpSimd builds the identity matrix and
    the block-diagonal mask; ScalarE preloads the Gelu ACT table.
  phase 2 (epilogue): mm1 -> gelu -> PE-transpose -> mm2 -> gelu ->
    PE-transpose -> [A|Bv] matmul (+bias via ones-row matmul) -> outer
    product as one PE matmul against a block-diagonal Bv matrix (built with
    one masked VectorE multiply) -> sigmoid on (56, 224) -> DMA out.
"""

import numpy as np

import concourse.bass as bass
from concourse import mybir
from concourse.bass_utils import run_bass_kernel_spmd

B, C, HID, H, W = 32, 1024, 512, 56, 56
NCORES = 8
BS = B // NCORES          # 4 batches per core
ROWS = BS * C             # 4096 (b, c) rows per core
HW = H * W                # 3136
NBLK = ROWS // 128        # 32 row-blocks of 128
# per-DMA-tile block counts; small head tiles so the reduce engines start
# early, small tail so the final reduce is short
TILE_SIZES = [2, 2, 4, 4, 4, 4, 4, 4, 2, 1, 1]
assert sum(TILE_SIZES) == NBLK
NT = len(TILE_SIZES)
SLOT_BLKS = max(TILE_SIZES)   # buffer slot capacity (blocks)
NBUF = 4                      # x buffer ring slots
F32 = mybir.dt.float32
BF16 = mybir.dt.bfloat16


def build_bass(gelu_fn=None, debug_taps=False) -> bass.Bass:
    # gelu_fn override exists only so CoreSim (which lacks a Gelu impl) can
    # check the dataflow with a substitute activation.
    if gelu_fn is None:
        gelu_fn = mybir.ActivationFunctionType.Gelu
    nc = bass.Bass()

    x_t = nc.dram_tensor("x", [ROWS, HW], BF16, kind="ExternalInput")
    w1_t = nc.dram_tensor("W1bf", [C, HID], BF16, kind="ExternalInput")
    w2_t = nc.dram_tensor("W2bf", [HID, C], BF16, kind="ExternalInput")
    wa_t = nc.dram_tensor("WAbf", [C, H], BF16, kind="ExternalInput")
    ba_t = nc.dram_tensor("bAbf", [H], BF16, kind="ExternalInput")
    wb_t = nc.dram_tensor("WBbf", [C, W], BF16, kind="ExternalInput")
    bb_t = nc.dram_tensor("bBbf", [W], BF16, kind="ExternalInput")
    out_t = nc.dram_tensor("out", [BS, HW], F32, kind="ExternalOutput")
    dbg = {}
    if debug_taps:
        for name, shape in [
            ("dbg_ysum", [128, NBLK]), ("dbg_ysum_bf", [128, NBLK]),
            ("dbg_h", [BS, HID]), ("dbg_yp", [BS, C]),
            ("dbg_ab", [BS, H + W]), ("dbg_bdiag", [BS, BS * W]),
            ("dbg_at", [H, BS * W]),
        ]:
            dbg[name] = nc.dram_tensor(name, shape, F32, kind="ExternalOutput")

    # x row r = b*C + c; block j = r // 128 = b*8 + c_chunk.  Tile n covers
    # blocks [off_n, off_n + s_n); ysum column j holds sum of row block j.
    x_blk = x_t[:, :].rearrange("(j p) m -> j p m", p=128)
    offs = [sum(TILE_SIZES[:n]) for n in range(NT)]

    # Block-reduce ownership: DVE uses scalar_tensor_tensor as a fused
    # pairwise-add + accumulate (~2.15us/block measured); ACT reduces via
    # activation(Copy, accum_out) (~3.5us/block).  20:12 split matches the
    # measured rates; the last block goes to the faster DVE lane.
    OWNER = ["A" if j % 8 in (1, 4, 6) else "D" for j in range(NBLK)]
    # cumulative per-owner block counts through tile t
    cumD = [sum(1 for j in range(offs[t] + TILE_SIZES[t]) if OWNER[j] == "D")
            for t in range(NT)]
    cumA = [sum(1 for j in range(offs[t] + TILE_SIZES[t]) if OWNER[j] == "A")
            for t in range(NT)]

    # ---- SBUF ----
    x_sb = nc.alloc_sbuf_tensor("x_sb", [128, NBUF, SLOT_BLKS, HW], BF16)
    # throwaway elementwise outputs of the accumulate-reduces (only
    # accum_out matters); per-engine ops serialize so one scratch each
    ascr_sb = nc.alloc_sbuf_tensor("ascr_sb", [128, HW], BF16)
    dscr_sb = nc.alloc_sbuf_tensor("dscr_sb", [128, HW // 2], BF16)
    ysum_sb = nc.alloc_sbuf_tensor("ysum_sb", [128, NBLK], F32)
    ysum_bf = nc.alloc_sbuf_tensor("ysum_bf", [128, NBLK], BF16)
    w1_sb = nc.alloc_sbuf_tensor("w1_sb", [128, C // 128, HID], BF16)
    w2_sb = nc.alloc_sbuf_tensor("w2_sb", [128, HID // 128, C], BF16)
    wab_sb = nc.alloc_sbuf_tensor("wab_sb", [128, C // 128, H + W], BF16)
    bab_sb = nc.alloc_sbuf_tensor("bab_sb", [1, H + W], BF16)
    ident_sb = nc.alloc_sbuf_tensor("ident_sb", [128, 128], BF16)
    ones_sb = nc.alloc_sbuf_tensor("ones_sb", [1, BS], BF16)
    mask_sb = nc.alloc_sbuf_tensor("mask_sb", [BS, BS, W], BF16)
    h_sb = nc.alloc_sbuf_tensor("h_sb", [BS, HID], BF16)
    hT_sb = nc.alloc_sbuf_tensor("hT_sb", [128, (HID // 128) * BS], BF16)
    yp_sb = nc.alloc_sbuf_tensor("yp_sb", [BS, C], BF16)
    ypT_sb = nc.alloc_sbuf_tensor("ypT_sb", [128, (C // 128) * BS], BF16)
    ab_sb = nc.alloc_sbuf_tensor("ab_sb", [BS, H + W], BF16)
    bdiag_sb = nc.alloc_sbuf_tensor("bdiag_sb", [BS, BS, W], BF16)
    attn_sb = nc.alloc_sbuf_tensor("attn_sb", [H, BS, W], F32)
    scr_sb = nc.alloc_sbuf_tensor("scr_sb", [1, 1], F32)

    # ---- PSUM (each tensor gets its own 2KB bank; 7 of 8 banks used) ----
    ps_h = nc.alloc_psum_tensor("ps_h", [BS, HID], F32)
    ps_yp1 = nc.alloc_psum_tensor("ps_yp1", [BS, C // 2], F32)
    ps_yp2 = nc.alloc_psum_tensor("ps_yp2", [BS, C // 2], F32)
    ps_ab = nc.alloc_psum_tensor("ps_ab", [BS, H + W], F32)
    ps_at = nc.alloc_psum_tensor("ps_at", [H, BS, W], F32)
    ps_warm = nc.alloc_psum_tensor("ps_warm", [BS, 128], F32)
    # two transpose scratch banks, ping-pong so PE-write and DVE-read never
    # touch the same PSUM bank concurrently (HW fault otherwise)
    tp_banks = [
        nc.alloc_psum_tensor("tp_a", [128, BS], BF16),
        nc.alloc_psum_tensor("tp_b", [128, BS], BF16),
    ]

    # ---- semaphores ----
    # One semaphore per DMA: with several DMAs in flight on one counting
    # semaphore, per-SDMA-engine increments can interleave and a cumulative
    # wait_ge would not imply the earlier DMA fully landed.
    xdma_sems = [nc.alloc_semaphore(f"xdma_sem{n}") for n in range(NT)]
    w_sems = [nc.alloc_semaphore(f"w_sem{i}") for i in range(6)]
    id_sem = nc.alloc_semaphore("id_sem")
    ones_sem = nc.alloc_semaphore("ones_sem")
    red_d = nc.alloc_semaphore("red_d")
    red_a = nc.alloc_semaphore("red_a")
    pe_sem = nc.alloc_semaphore("pe_sem")
    act_sem = nc.alloc_semaphore("act_sem")
    dve_sem = nc.alloc_semaphore("dve_sem")
    out_sem = nc.alloc_semaphore("out_sem")
    out2_sem = nc.alloc_semaphore("out2_sem")

    NCC = C // 128    # 8
    NQH = HID // 128  # 4


    # lhsT for chunk cc: ysum columns j = b*8 + cc, b = 0..3
    ysum_r = ysum_bf[:, :].rearrange("p (b c) -> p c b", b=BS)

    # Single Block: a Block exit drains every engine (including waiting for
    # outstanding SWDGE DMA completions) and cross-syncs, so phase
    # boundaries inside the kernel would serialize the pipeline.  All
    # ordering is via explicit semaphores instead.
    with nc.Block() as blk:

        @blk.sync
        def _(sync):
            w_loads = [
                (w1_sb[:, :, :],
                 w1_t[:, :].rearrange("(n p) h -> p n h", p=128)),
                (w2_sb[:, :, :],
                 w2_t[:, :].rearrange("(n p) h -> p n h", p=128)),
                (wab_sb[:, :, 0:H],
                 wa_t[:, :].rearrange("(n p) h -> p n h", p=128)),
                (wab_sb[:, :, H : H + W],
                 wb_t[:, :].rearrange("(n p) h -> p n h", p=128)),
                (bab_sb[0:1, 0:H], ba_t[None, :]),
                (bab_sb[0:1, H : H + W], bb_t[None, :]),
            ]
            for n in range(NT):
                if n >= NBUF:
                    # slot reuse: all blocks of tile n-NBUF must be reduced
                    sync.wait_ge(red_d, cumD[n - NBUF])
                    sync.wait_ge(red_a, cumA[n - NBUF])
                sync.dma_start(
                    out=x_sb[:, n % NBUF, 0 : TILE_SIZES[n], :],
                    in_=x_blk[offs[n] : offs[n] + TILE_SIZES[n]].rearrange(
                        "j p m -> p j m"
                    ),
                ).then_inc(xdma_sems[n], 16)
                if n == 0:
                    # weights ride the same HWDGE queue right behind tile 0:
                    # HWDGE descriptors avoid the SWDGE round-robin penalty,
                    # and the loads are done long before mm1 needs them
                    for i, (dst, src) in enumerate(w_loads):
                        sync.dma_start(out=dst, in_=src).then_inc(w_sems[i], 16)
            out_r = out_t[:, :].rearrange("b (h w) -> h b w", h=H)
            sync.wait_ge(act_sem, 5)
            sync.dma_start(
                out=out_r[0 : 32], in_=attn_sb[0 : 32, :, :]
            ).then_inc(out_sem, 16)
            sync.wait_ge(act_sem, 6)
            sync.dma_start(
                out=out_r[32 : H], in_=attn_sb[32 : H, :, :]
            ).then_inc(out2_sem, 16)
            sync.wait_ge(out_sem, 16)
            sync.wait_ge(out2_sem, 16)

        @blk.vector
        def _(vec):
            vec.memset(ones_sb[:, :], 1.0).then_inc(ones_sem, 1)
            for n in range(NT):
                if not any(OWNER[offs[n] + k] == "D" for k in range(TILE_SIZES[n])):
                    continue
                vec.wait_ge(xdma_sems[n], 16)
                for k in range(TILE_SIZES[n]):
                    j = offs[n] + k
                    if OWNER[j] != "D":
                        continue
                    nc.vector.scalar_tensor_tensor(
                        out=dscr_sb[:, :],
                        in0=x_sb[:, n % NBUF, k, 0 : HW // 2],
                        scalar=0.0,
                        in1=x_sb[:, n % NBUF, k, HW // 2 : HW],
                        op0=mybir.AluOpType.add,
                        op1=mybir.AluOpType.add,
                        accum_out=ysum_sb[:, j : j + 1],
                    ).then_inc(red_d, 1)
            for q in range(NQH):
                vec.wait_ge(pe_sem, 9 + q)
                nc.vector.tensor_copy(
                    out=hT_sb[:, q * BS : (q + 1) * BS],
                    in_=tp_banks[q % 2][:, :],
                ).then_inc(dve_sem, 1)
            for q in range(NCC):
                vec.wait_ge(pe_sem, 21 + q)
                nc.vector.tensor_copy(
                    out=ypT_sb[:, q * BS : (q + 1) * BS],
                    in_=tp_banks[q % 2][:, :],
                ).then_inc(dve_sem, 1)
            vec.wait_ge(pe_sem, 37)
            nc.vector.tensor_copy(
                out=ab_sb[:, :], in_=ps_ab[:, :]
            ).then_inc(dve_sem, 1)
            vec.wait_ge(dve_sem, 1 + NQH + NCC)
            vec.wait_ge(id_sem, 4)
            # bdiag[b, bb, w] = Bv[b, w] * (b == bb)
            b_sl = ab_sb[:, H : H + W]
            b_bc = bass.AP(
                tensor=b_sl.tensor, offset=b_sl.offset,
                ap=[b_sl.ap[0], [0, BS], [b_sl.ap[1][0], W]],
            )
            nc.vector.tensor_mul(
                out=bdiag_sb[:, :, :], in0=b_bc, in1=mask_sb[:, :, :]
            ).then_inc(dve_sem, 1)

        @blk.gpsimd
        def _(gpsimd):
            # the 8 Q7 cores don't serialize consecutive Pool ops, so sync
            # memset -> affine_select explicitly
            gpsimd.memset(ident_sb[:, :], 0.0).then_inc(id_sem, 1)
            gpsimd.memset(mask_sb[:, :, :], 0.0).then_inc(id_sem, 1)
            gpsimd.wait_ge(id_sem, 2)
            gpsimd.affine_select(
                out=ident_sb[:, :],
                in_=ident_sb[:, :],
                compare_op=mybir.AluOpType.not_equal,
                fill=1.0,
                base=0,
                pattern=[[-1, 128]],
                channel_multiplier=1,
            ).then_inc(id_sem, 1)
            # mask[p, bb, w] = (p == bb) ? 1 : 0
            gpsimd.affine_select(
                out=mask_sb[:, :, :],
                in_=mask_sb[:, :, :],
                compare_op=mybir.AluOpType.not_equal,
                fill=1.0,
                base=0,
                pattern=[[-1, BS], [0, W]],
                channel_multiplier=1,
            ).then_inc(id_sem, 1)
            if debug_taps:
                gpsimd.wait_ge(act_sem, 6)
                taps = [
                    (dbg["dbg_ysum"], ysum_sb[:, :]),
                    (dbg["dbg_ysum_bf"], ysum_bf[:, :]),
                    (dbg["dbg_h"], h_sb[:, :]),
                    (dbg["dbg_yp"], yp_sb[:, :]),
                    (dbg["dbg_ab"], ab_sb[:, :]),
                    (dbg["dbg_bdiag"],
                     bdiag_sb[:, :, :].rearrange("b bb w -> b (bb w)")),
                    (dbg["dbg_at"],
                     attn_sb[:, :, :].rearrange("h b w -> h (b w)")),
                ]
                dbg_sem = nc.alloc_semaphore("dbg_sem")
                for i, (dst, src_ap) in enumerate(taps):
                    gpsimd.dma_start(out=dst[:, :], in_=src_ap).then_inc(
                        dbg_sem, 16
                    )
                    gpsimd.wait_ge(dbg_sem, 16 * (i + 1))

        # PE ticks (every non-dummy PE op +1 on pe_sem):
        #  1..8  mm1        9..12  transpose h    13..20 mm2 (yp1@19, yp2@20)
        #  21..28 tr yp     29..37 mm3 + bias     38 outer product
        # ACT: ysum cast 1, gelu_h 2, gelu_yp1 3, gelu_yp2 4, sigmoid 5
        # DVE: hT copies 1..4, ypT copies 5..12, ab copy 13, bdiag mul 14
        # id_sem: ident memset/select + mask memset/select (GpSimd) + ones

        @blk.tensor
        def _(pe):
            pe.wait_ge(id_sem, 4)
            pe.wait_ge(ones_sem, 1)
            # warm the PE clock (HAM): opportunistic burst near stream
            # end, then a full 3.4us HAM-window burst paced by the last
            # x-DMA landing -- it runs exactly during the final reduce +
            # cast, so mm1 always starts warm and undelayed
            pe.wait_ge(red_d, cumD[NT - 2])
            for _i in range(24):
                nc.tensor.matmul(
                    ps_warm[:, :], ident_sb[:, 0:BS], ident_sb[:, :],
                    start=True, stop=True,
                )
            pe.wait_ge(xdma_sems[NT - 1], 16)
            for _i in range(16):
                nc.tensor.matmul(
                    ps_warm[:, :], ident_sb[:, 0:BS], ident_sb[:, :],
                    start=True, stop=True,
                )
            pe.wait_ge(act_sem, 1)
            pe.wait_ge(w_sems[0], 16)
            for cc in range(NCC):
                nc.tensor.matmul(
                    ps_h[:, :],
                    ysum_r[:, cc, :],
                    w1_sb[:, cc, :],
                    start=(cc == 0),
                    stop=(cc == NCC - 1),
                ).then_inc(pe_sem, 1)
            pe.wait_ge(act_sem, 2)
            for q in range(NQH):
                if q >= 2:
                    pe.wait_ge(dve_sem, q - 1)
                nc.tensor.transpose(
                    tp_banks[q % 2][:, :],
                    h_sb[:, q * 128 : (q + 1) * 128],
                    ident_sb[:BS, :BS],
                ).then_inc(pe_sem, 1)
            pe.wait_ge(w_sems[1], 16)
            for q in range(NQH):
                pe.wait_ge(dve_sem, q + 1)
                lhsT = hT_sb[:, q * BS : (q + 1) * BS]
                nc.tensor.matmul(
                    ps_yp1[:, :], lhsT, w2_sb[:, q, 0 : C // 2],
                    start=(q == 0), stop=(q == NQH - 1),
                ).then_inc(pe_sem, 1)
                nc.tensor.matmul(
                    ps_yp2[:, :], lhsT, w2_sb[:, q, C // 2 : C],
                    start=(q == 0), stop=(q == NQH - 1),
                ).then_inc(pe_sem, 1)
            pe.wait_ge(act_sem, 3)
            for q in range(NCC):
                if q == NQH:
                    pe.wait_ge(act_sem, 4)
                if q >= 2:
                    pe.wait_ge(dve_sem, q + 3)
                nc.tensor.transpose(
                    tp_banks[q % 2][:, :],
                    yp_sb[:, q * 128 : (q + 1) * 128],
                    ident_sb[:BS, :BS],
                ).then_inc(pe_sem, 1)
            pe.wait_ge(w_sems[2], 16)
            pe.wait_ge(w_sems[3], 16)
            for cc in range(NCC):
                pe.wait_ge(dve_sem, NQH + 1 + cc)
                nc.tensor.matmul(
                    ps_ab[:, :],
                    ypT_sb[:, cc * BS : (cc + 1) * BS],
                    wab_sb[:, cc, :],
                    start=(cc == 0),
                    stop=False,
                ).then_inc(pe_sem, 1)
            pe.wait_ge(w_sems[4], 16)
            pe.wait_ge(w_sems[5], 16)
            nc.tensor.matmul(
                ps_ab[:, :], ones_sb[:, :], bab_sb[:, :],
                start=False, stop=True,
            ).then_inc(pe_sem, 1)
            # outer products: out[h, (b w)] = sum_b' A[b', h] * bdiag[b', (b w)]
            pe.wait_ge(dve_sem, 2 + NQH + NCC)
            nc.tensor.matmul(
                ps_at[:, :, :].rearrange("h b w -> h (b w)"),
                ab_sb[:, 0:H],
                bdiag_sb[:, :, :].rearrange("b bb w -> b (bb w)"),
                start=True, stop=True,
            ).then_inc(pe_sem, 1)

        @blk.scalar
        def _(act):
            # dummy activation so walrus loads the Gelu ACT table here, early
            # and in the same basic block as the real gelus (no reload)
            zero = nc.const_aps.aps[(F32, 0.0)]
            nc.scalar.activation(scr_sb[0:1, :], zero[0:1, :], gelu_fn)
            # ACT's share of the block reduces (activation Copy + accum_out)
            for n in range(NT):
                if not any(OWNER[offs[n] + k] == "A" for k in range(TILE_SIZES[n])):
                    continue
                act.wait_ge(xdma_sems[n], 16)
                for k in range(TILE_SIZES[n]):
                    j = offs[n] + k
                    if OWNER[j] != "A":
                        continue
                    nc.scalar.activation(
                        out=ascr_sb[:, :],
                        in_=x_sb[:, n % NBUF, k, :],
                        func=mybir.ActivationFunctionType.Copy,
                        accum_out=ysum_sb[:, j : j + 1],
                    ).then_inc(red_a, 1)
            act.wait_ge(red_d, cumD[NT - 1])
            nc.scalar.copy(
                out=ysum_bf[:, :], in_=ysum_sb[:, :]
            ).then_inc(act_sem, 1)
            act.wait_ge(pe_sem, NCC)
            nc.scalar.activation(
                h_sb[:, :], ps_h[:, :], gelu_fn, scale=1.0 / HW
            ).then_inc(act_sem, 1)
            act.wait_ge(pe_sem, 8 + NQH + 2 * NQH - 1)   # yp1 done @19
            nc.scalar.activation(
                yp_sb[:, 0 : C // 2], ps_yp1[:, :], gelu_fn
            ).then_inc(act_sem, 1)
            act.wait_ge(pe_sem, 8 + NQH + 2 * NQH)       # yp2 done @20
            nc.scalar.activation(
                yp_sb[:, C // 2 : C], ps_yp2[:, :], gelu_fn
            ).then_inc(act_sem, 1)
            # dummy sigmoid so the ACT table switch happens off the
            # critical path, while the PE is still on transposes/mm3
            nc.scalar.activation(
                scr_sb[0:1, :], zero[0:1, :],
                mybir.ActivationFunctionType.Sigmoid,
            )
            # two halves so the first output DMA's HBM write receipt
            # overlaps the second half's sigmoid + transfer
            act.wait_ge(pe_sem, 38)
            nc.scalar.activation(
                attn_sb[0 : 32, :, :], ps_at[0 : 32, :, :],
                mybir.ActivationFunctionType.Sigmoid,
            ).then_inc(act_sem, 1)
            nc.scalar.activation(
                attn_sb[32 : H, :, :], ps_at[32 : H, :, :],
                mybir.ActivationFunctionType.Sigmoid,
            ).then_inc(act_sem, 1)

    return nc


_NC_CACHE: list = []


def run_on_hw(x, W1, W2, WA, bA, WB, bB, **spmd_kwargs):
    """Run the SPMD kernel; returns (full_output, BassKernelResults)."""
    import ml_dtypes

    bf = ml_dtypes.bfloat16
    # bf16 input stream: halves HBM traffic for the dominant x read; the
    # pooled-mean perturbation is ~0.6% of y's std, far inside tolerance
    x = np.ascontiguousarray(np.asarray(x, dtype=np.float32).astype(bf))
    weights = {
        "W1bf": np.ascontiguousarray(np.asarray(W1).astype(bf)),
        "W2bf": np.ascontiguousarray(np.asarray(W2).astype(bf)),
        "WAbf": np.ascontiguousarray(np.asarray(WA).astype(bf)),
        "bAbf": np.ascontiguousarray(np.asarray(bA).astype(bf)),
        "WBbf": np.ascontiguousarray(np.asarray(WB).astype(bf)),
        "bBbf": np.ascontiguousarray(np.asarray(bB).astype(bf)),
    }

    if not _NC_CACHE:
        _NC_CACHE.append(build_bass())
    nc = _NC_CACHE[0]

    in_maps = []
    for i in range(NCORES):
        shard = x[i * BS : (i + 1) * BS].reshape(ROWS, HW)
        in_maps.append({"x": shard, **weights})

    res = run_bass_kernel_spmd(
        nc, in_maps, core_ids=list(range(NCORES)), **spmd_kwargs
    )
    attn = np.concatenate([r["out"] for r in res.results], axis=0)  # (B, HW)
    return np.broadcast_to(attn.reshape(B, 1, H, W), (B, C, H, W)), res


def kernel(x, W1, W2, WA, bA, WB, bB):
    out, _ = run_on_hw(x, W1, W2, WA, bA, WB, bB)
    return out

